# revision 20
# baseline (speedup 1.0000x reference)
import os
import sys

import ml_dtypes
import numpy as np

if "/opt/trn_rl_repo" not in sys.path:
    sys.path.insert(0, "/opt/trn_rl_repo")

import concourse.bass as bass
import concourse.mybir as mybir
import concourse.tile as tile
from concourse import bacc, bass_utils
from concourse.bass import ds, ts

B, C, W, H, D = 4, 512, 2048, 4, 64
P = 128
CT = C // P  # 4 contraction tiles of 128 over channels
IT = W // P  # 16 row blocks over sequence
JT = W // 512  # 4 column chunks of 512 over sequence
ET = C // P  # 4 output-channel blocks
FP32 = mybir.dt.float32
BF16 = mybir.dt.bfloat16
F8 = mybir.dt.float8e4
E4M3 = ml_dtypes.float8_e4m3

# fp8 scaling bookkeeping:
#   wqs = 32*(Wq^T/sqrt(D)) both heads stacked -> scores s' = 1024*s
#   exp: p = exp(s'/1024 - ln 8) = e^s/8  (keeps e4m3 in normal range)
#   rsum_raw = sum_j p = S/8; rinv = 8/S
#   wv8 = 128*Wv^T -> vp = 128*v; vt8 = vp*rinv = 1024*v/S
#   ctx' = vt8 @ p = 128*ctx; residual rs = 256*x on even cores; host /128
QK_SCALE = 32.0
V_SCALE = 128.0
GAMMA = 128.0
ACT_SCALE = 1.0 / (QK_SCALE * QK_SCALE)
EXP_BIAS = -2.0794415416798357  # -ln(8)

_NC_CACHE = None
LAST_EXEC_NS = None
LAST_MEAN_EXEC_NS = None


def _build():
    nc = bacc.Bacc("TRN2", target_bir_lowering=False)
    x8_d = nc.dram_tensor("x8", (C, W), F8, kind="ExternalInput")
    xb_d = nc.dram_tensor("xb", (C, W), BF16, kind="ExternalInput")
    wqs_d = nc.dram_tensor("wqs", (C, P), F8, kind="ExternalInput")
    wks_d = nc.dram_tensor("wks", (C, P), F8, kind="ExternalInput")
    wv_d = nc.dram_tensor("wv", (2, C, C), F8, kind="ExternalInput")
    rs_d = nc.dram_tensor("rs", (P, 1), FP32, kind="ExternalInput")
    out_d = nc.dram_tensor("out", (C, W), FP32, kind="ExternalOutput")

    with tile.TileContext(nc) as tc:
        with (
            tc.tile_pool(name="sb", bufs=1) as sb,
            tc.tile_pool(name="ps", bufs=1, space="PSUM") as ps,
        ):
            x8_sb = sb.tile((P, CT, W), F8)
            xb_sb = sb.tile((P, CT, W), BF16)
            wqs_sb = sb.tile((P, CT, P), F8)
            wks_sb = sb.tile((P, CT, P), F8)
            wv_sb = sb.tile((P, 2, CT, C), F8)
            rs_sb = sb.tile((P, 1), FP32)
            eb_sb = sb.tile((P, 1), FP32)
            warm_sb = sb.tile((P, 512), BF16)
            outa = sb.tile((P, ET, W), FP32)
            qb_sb = sb.tile((P, W), BF16)  # rows 0-63 h0 q, 64-127 h1 q
            kb_sb = sb.tile((P, W), BF16)
            v1_sb = sb.tile((P, IT, C), BF16)  # head1 v^T staging (bf16)
            q1dup = sb.tile((D, W), BF16)  # h1 q duplicated to rows 0-63
            k1dup = sb.tile((D, W), BF16)
            p_sb = sb.tile((P, 2, IT, JT, 512), F8)
            vt8_sb = sb.tile((P, 2, IT, C), F8)
            sums = sb.tile((P, 2, IT, 2), FP32)  # exp accums per (h, it, j2)
            rsum = sb.tile((P, 2, IT), FP32)
            rinv = sb.tile((P, 2, IT), FP32)

            qs = [nc.sync, nc.gpsimd, nc.scalar]
            # input DMAs: contiguous chunks spread over all three queues;
            # first-needed first (wqs/wks + x8 low columns gate compute)
            nc.gpsimd.memset(warm_sb[:], 0.0)
            nc.gpsimd.memset(eb_sb[:], EXP_BIAS)
            nc.sync.dma_start(x8_sb[:, 0, 0:512], x8_d[ts(0, P), 0:512])
            nc.scalar.dma_start(x8_sb[:, 1, 0:512], x8_d[ts(1, P), 0:512])
            nc.sync.dma_start(x8_sb[:, 2, 0:512], x8_d[ts(2, P), 0:512])
            nc.scalar.dma_start(x8_sb[:, 3, 0:512], x8_d[ts(3, P), 0:512])
            nc.gpsimd.dma_start(wqs_sb[:, 0], wqs_d[ts(0, P), :])
            nc.gpsimd.dma_start(wqs_sb[:, 1], wqs_d[ts(1, P), :])
            nc.gpsimd.dma_start(wqs_sb[:, 2], wqs_d[ts(2, P), :])
            nc.gpsimd.dma_start(wqs_sb[:, 3], wqs_d[ts(3, P), :])
            nc.sync.dma_start(wks_sb[:, 0], wks_d[ts(0, P), :])
            nc.scalar.dma_start(wks_sb[:, 1], wks_d[ts(1, P), :])
            nc.sync.dma_start(wks_sb[:, 2], wks_d[ts(2, P), :])
            nc.scalar.dma_start(wks_sb[:, 3], wks_d[ts(3, P), :])
            for ct in range(CT):
                [nc.sync, nc.scalar][ct % 2].dma_start(
                    x8_sb[:, ct, 512:W], x8_d[ts(ct, P), 512:W]
                )
            for ct in range(CT):
                nc.sync.dma_start(wv_sb[:, 0, ct], wv_d[0, ts(ct, P), :])
            for ct in range(CT):
                nc.scalar.dma_start(wv_sb[:, 1, ct], wv_d[1, ts(ct, P), :])
            nc.gpsimd.dma_start(rs_sb[:], rs_d[:])
            xb_src = xb_d[:].rearrange("(ct p) w -> p ct w", p=P)
            for half in range(2):
                nc.gpsimd.dma_start(
                    xb_sb[:, ds(2 * half, 2)], xb_src[:, ds(2 * half, 2)]
                )

            DR = mybir.MatmulPerfMode.DoubleRow

            # HAM warm-up: dummy matmuls so the PE clock is at 8/8 when real
            # work (gated on DMA arrival) begins
            wp = ps.tile((P, 512), FP32, tag="gp", bufs=2, name="wp")
            for _ in range(12):
                nc.tensor.matmul(wp[:], warm_sb[:, 0:128], warm_sb[:])

            def qk_nt(nt):
                # both heads' q (or k) in one stacked M=128 chain; k chains
                # use the otherwise-idle "sc" banks so the gp rotation never
                # gates phase 0
                qp = ps.tile((P, 512), FP32, tag="gp", bufs=2, name="qp")
                for cc in range(CT // 2):
                    nc.tensor.matmul(
                        qp[:],
                        wqs_sb[:, ds(2 * cc, 2), :],
                        x8_sb[:, ds(2 * cc, 2), ts(nt, 512)],
                        start=(cc == 0),
                        stop=(cc == CT // 2 - 1),
                        perf_mode=DR,
                    )
                nc.scalar.copy(qb_sb[:, ts(nt, 512)], qp[:])
                kt = ps.tile((P, 2, 512), FP32, tag="sc", bufs=3, name="sp")
                kp = kt[:, 0]
                for cc in range(CT // 2):
                    nc.tensor.matmul(
                        kp,
                        wks_sb[:, ds(2 * cc, 2), :],
                        x8_sb[:, ds(2 * cc, 2), ts(nt, 512)],
                        start=(cc == 0),
                        stop=(cc == CT // 2 - 1),
                        perf_mode=DR,
                    )
                nc.vector.tensor_copy(kb_sb[:, ts(nt, 512)], kp)

            def sc_exp(h, it):
                # h0 in array rows 0-63, h1 in rows 64-127 (row-group MMs)
                lo, hi = (0, D) if h == 0 else (D, P)
                for j2 in range(2):
                    sp = ps.tile((P, 2, 512), FP32, tag="sc", bufs=3, name="sp")
                    for jh in range(2):
                        nc.tensor.matmul(
                            sp[:, jh],
                            qb_sb[lo:hi, ts(it, P)],
                            kb_sb[lo:hi, ds(j2 * 1024 + jh * 512, 512)],
                        )
                    nc.scalar.activation(
                        p_sb[:, h, it, ds(2 * j2, 2)],
                        sp[:],
                        mybir.ActivationFunctionType.Exp,
                        bias=eb_sb[:],
                        scale=ACT_SCALE,
                        accum_out=sums[:, h, it, ds(j2, 1)],
                    )

            def sc_exp_h1_paired(it):
                # h1 scores with j2=0 in rows 64-127 and j2=1 in rows 0-63
                # (via the duplicated q/k) -> adjacent MMs in different row
                # groups run concurrently on the PE array
                sp0 = ps.tile((P, 2, 512), FP32, tag="sc", bufs=3, name="sp")
                sp1 = ps.tile((P, 2, 512), FP32, tag="sc", bufs=3, name="sp")
                for jh in range(2):
                    nc.tensor.matmul(
                        sp0[:, jh],
                        qb_sb[D:P, ts(it, P)],
                        kb_sb[D:P, ds(jh * 512, 512)],
                    )
                    nc.tensor.matmul(
                        sp1[:, jh],
                        q1dup[:, ts(it, P)],
                        k1dup[:, ds(1024 + jh * 512, 512)],
                    )
                for j2, sp in ((0, sp0), (1, sp1)):
                    nc.scalar.activation(
                        p_sb[:, 1, it, ds(2 * j2, 2)],
                        sp[:],
                        mybir.ActivationFunctionType.Exp,
                        bias=eb_sb[:],
                        scale=ACT_SCALE,
                        accum_out=sums[:, 1, it, ds(j2, 1)],
                    )

            def vt_mm(h, it):
                vp = ps.tile((P, 512), FP32, tag="gp", bufs=2, name="vp")
                for cc in range(CT // 2):
                    nc.tensor.matmul(
                        vp[:],
                        x8_sb[:, ds(2 * cc, 2), ts(it, P)],
                        wv_sb[:, h, ds(2 * cc, 2), :],
                        start=(cc == 0),
                        stop=(cc == CT // 2 - 1),
                        perf_mode=DR,
                    )
                return vp

            def rinv_it(h, it):
                nc.vector.tensor_add(
                    rsum[:, h, ds(it, 1)],
                    sums[:, h, it, ds(0, 1)],
                    sums[:, h, it, ds(1, 1)],
                )
                nc.vector.reciprocal(rinv[:, h, ds(it, 1)], rsum[:, h, ds(it, 1)])

            def ctx_chunk(h, et, jt, dma_out):
                cp = ps.tile((P, 512), FP32, tag="gp", bufs=2, name="cp")
                for kk in range(IT // 2):
                    nc.tensor.matmul(
                        cp[:],
                        vt8_sb[:, h, ds(2 * kk, 2), ts(et, P)],
                        p_sb[:, h, ds(2 * kk, 2), jt],
                        start=(kk == 0),
                        stop=(kk == IT // 2 - 1),
                        perf_mode=DR,
                    )
                nc.vector.tensor_add(
                    outa[:, et, ts(jt, 512)], outa[:, et, ts(jt, 512)], cp[:]
                )
                if dma_out:
                    eng = qs[(et * JT + jt) % 3]
                    eng.dma_start(
                        out_d[ts(et, P), ts(jt, 512)], outa[:, et, ts(jt, 512)]
                    )

            # phase 0: stacked qk projection (both heads per chain)
            for nt in range(JT):
                qk_nt(nt)

            # phase 1: ACT-bound exp h0; PE also precomputes v^T h1
            for it in range(IT):
                sc_exp(0, it)
                vp0 = vt_mm(0, it)
                rinv_it(0, it)
                nc.vector.tensor_scalar_mul(
                    vt8_sb[:, 0, it], vp0[:], rinv[:, 0, ds(it, 1)]
                )
                vp1 = vt_mm(1, it)
                nc.vector.tensor_copy(v1_sb[:, it], vp1[:])
                if it == 14:
                    nc.vector.tensor_scalar_mul(outa[:, 0], xb_sb[:, 0], rs_sb[:])

            # phase 2: ACT exp h1; PE ctx h0 lagged one it behind the scores
            for it in range(IT):
                sc_exp(1, it)
                rinv_it(1, it)
                nc.vector.tensor_scalar_mul(
                    vt8_sb[:, 1, it], v1_sb[:, it], rinv[:, 1, ds(it, 1)]
                )
                if it in (2, 6, 10):
                    ct = it // 4 + 1
                    nc.vector.tensor_scalar_mul(outa[:, ct], xb_sb[:, ct], rs_sb[:])
                if it >= 1:
                    t = it - 1
                    ctx_chunk(0, t // JT, t % JT, dma_out=False)
            ctx_chunk(0, 3, 3, dma_out=False)

            # phase 3: ctx h1, ACT idle
            for et in range(ET):
                for jt in range(JT):
                    ctx_chunk(1, et, jt, dma_out=True)

    nc.finalize()
    return nc


def kernel(x, Wq, bq, Wk, bk, Wv, bv):
    global _NC_CACHE, LAST_EXEC_NS, LAST_MEAN_EXEC_NS
    x = np.ascontiguousarray(np.asarray(x, dtype=np.float32))
    Wq = np.asarray(Wq, dtype=np.float32)
    Wk = np.asarray(Wk, dtype=np.float32)
    Wv = np.asarray(Wv, dtype=np.float32)
    scale = np.float32(D ** -0.5)

    if _NC_CACHE is None:
        _NC_CACHE = _build()
    nc = _NC_CACHE

    x8 = x.astype(E4M3)
    xb = x.astype(ml_dtypes.bfloat16)

    # core c -> batch c//2, head pair c%2 (heads 2p, 2p+1)
    # wqs/wks: both heads of the pair stacked on the output axis -> [C, 128]
    wqs_pair = []
    wks_pair = []
    wv_pair = []
    for pair in range(2):
        hs = [2 * pair, 2 * pair + 1]
        wqs_pair.append(
            np.ascontiguousarray(
                np.concatenate(
                    [Wq[h].T * (QK_SCALE * scale) for h in hs], axis=1
                ).astype(E4M3)
            )
        )
        wks_pair.append(
            np.ascontiguousarray(
                np.concatenate([Wk[h].T * QK_SCALE for h in hs], axis=1).astype(E4M3)
            )
        )
        wv_pair.append(
            np.ascontiguousarray(
                (np.stack([Wv[h].T for h in hs]) * V_SCALE).astype(E4M3)
            )
        )

    in_maps = []
    for c in range(8):
        b, pair = c // 2, c % 2
        in_maps.append(
            {
                "x8": x8[b],
                "xb": xb[b],
                "wqs": wqs_pair[pair],
                "wks": wks_pair[pair],
                "wv": wv_pair[pair],
                "rs": np.full(
                    (P, 1), 2.0 * GAMMA if pair == 0 else 0.0, dtype=np.float32
                ),
            }
        )

    res = bass_utils.run_bass_kernel_spmd(nc, in_maps, core_ids=list(range(8)))
    LAST_EXEC_NS = res.exec_time_ns
    LAST_MEAN_EXEC_NS = res.mean_exec_time_ns

    out = np.empty((B, C, W), dtype=np.float32)
    inv_g = np.float32(1.0 / GAMMA)
    for b in range(B):
        out[b] = (res.results[2 * b]["out"] + res.results[2 * b + 1]["out"]) * inv_g
    return out


# revision 25
# speedup vs baseline: 1.2051x; 1.2051x over previous
import os
import sys

import ml_dtypes
import numpy as np

if "/opt/trn_rl_repo" not in sys.path:
    sys.path.insert(0, "/opt/trn_rl_repo")

import concourse.bass as bass
import concourse.mybir as mybir
import concourse.tile as tile
from concourse import bacc, bass_utils
from concourse.bass import ds, ts

B, C, W, H, D = 4, 512, 2048, 4, 64
P = 128
CT = C // P  # 4 contraction tiles of 128 over channels
IT = W // P  # 16 row blocks over sequence
JT = W // 512  # 4 column chunks of 512 over sequence
ET = C // P  # 4 output-channel blocks
FP32 = mybir.dt.float32
BF16 = mybir.dt.bfloat16
F8 = mybir.dt.float8e4
E4M3 = ml_dtypes.float8_e4m3

# fp8 scaling bookkeeping:
#   wqs = 32*(Wq^T/sqrt(D)) both heads stacked -> scores s' = 1024*s
#   exp: p = exp(s'/1024 - ln 8) = e^s/8  (keeps e4m3 in normal range)
#   rsum_raw = sum_j p = S/8; rinv = 8/S
#   wv8 = 128*Wv^T -> vp = 128*v; vt8 = vp*rinv = 1024*v/S
#   ctx' = vt8 @ p = 128*ctx; residual rs = 256*x on even cores; host /128
QK_SCALE = 32.0
V_SCALE = 128.0
GAMMA = 128.0
ACT_SCALE = 1.0 / (QK_SCALE * QK_SCALE)
EXP_BIAS = -2.0794415416798357  # -ln(8)

_NC_CACHE = None
LAST_EXEC_NS = None
LAST_MEAN_EXEC_NS = None


def _build():
    nc = bacc.Bacc("TRN2", target_bir_lowering=False)
    x8_d = nc.dram_tensor("x8", (C, W), F8, kind="ExternalInput")
    xb_d = nc.dram_tensor("xb", (C, W), BF16, kind="ExternalInput")
    wqs_d = nc.dram_tensor("wqs", (C, P), F8, kind="ExternalInput")
    wks_d = nc.dram_tensor("wks", (C, P), F8, kind="ExternalInput")
    wv_d = nc.dram_tensor("wv", (2, C, C), F8, kind="ExternalInput")
    rs_d = nc.dram_tensor("rs", (P, 1), FP32, kind="ExternalInput")
    out_d = nc.dram_tensor("out", (C, W), FP32, kind="ExternalOutput")

    with tile.TileContext(nc) as tc:
        with (
            tc.tile_pool(name="sb", bufs=1) as sb,
            tc.tile_pool(name="ps", bufs=1, space="PSUM") as ps,
        ):
            x8_sb = sb.tile((P, CT, W), F8)
            xb_sb = sb.tile((P, CT, W), BF16)
            wqs_sb = sb.tile((P, CT, P), F8)
            wks_sb = sb.tile((P, CT, P), F8)
            wv_sb = sb.tile((P, 2, CT, C), F8)
            rs_sb = sb.tile((P, 1), FP32)
            eb_sb = sb.tile((P, 1), FP32)
            warm_sb = sb.tile((P, 512), BF16)
            outa = sb.tile((P, ET, W), FP32)
            qb_sb = sb.tile((P, W), BF16)  # rows 0-63 h0 q, 64-127 h1 q
            kb_sb = sb.tile((P, W), BF16)
            v1_sb = sb.tile((P, IT, C), BF16)  # head1 v^T staging (bf16)
            p_sb = sb.tile((P, 2, IT, JT, 512), F8)
            vt8_sb = sb.tile((P, 2, IT, C), F8)
            sums = sb.tile((P, 2, IT, 2), FP32)  # exp accums per (h, it, j2)
            rsum = sb.tile((P, 2, IT), FP32)
            rinv = sb.tile((P, 2, IT), FP32)

            qs = [nc.sync, nc.gpsimd, nc.scalar]
            # input DMAs: contiguous chunks spread over all three queues;
            # first-needed first (wqs/wks + x8 low columns gate compute)
            nc.gpsimd.memset(warm_sb[:], 0.0)
            nc.gpsimd.memset(eb_sb[:], EXP_BIAS)
            for ct in range(CT):
                nc.sync.dma_start(wqs_sb[:, ct], wqs_d[ts(ct, P), :])
            for ct in range(CT):
                nc.scalar.dma_start(wks_sb[:, ct], wks_d[ts(ct, P), :])
            for ct in range(CT):
                nc.gpsimd.dma_start(x8_sb[:, ct, 0:512], x8_d[ts(ct, P), 0:512])
            for ct in range(CT):
                [nc.sync, nc.scalar][ct % 2].dma_start(
                    x8_sb[:, ct, 512:W], x8_d[ts(ct, P), 512:W]
                )
            for ct in range(CT):
                nc.sync.dma_start(wv_sb[:, 0, ct], wv_d[0, ts(ct, P), :])
            for ct in range(CT):
                nc.scalar.dma_start(wv_sb[:, 1, ct], wv_d[1, ts(ct, P), :])
            nc.gpsimd.dma_start(rs_sb[:], rs_d[:])
            xb_src = xb_d[:].rearrange("(ct p) w -> p ct w", p=P)
            for half in range(2):
                nc.gpsimd.dma_start(
                    xb_sb[:, ds(2 * half, 2)], xb_src[:, ds(2 * half, 2)]
                )

            DR = mybir.MatmulPerfMode.DoubleRow

            # HAM warm-up: dummy matmuls so the PE clock is at 8/8 when real
            # work (gated on DMA arrival) begins
            wp = ps.tile((P, 512), FP32, tag="gp", bufs=2, name="wp")
            for _ in range(9):
                nc.tensor.matmul(wp[:], warm_sb[:, 0:128], warm_sb[:])

            def qk_nt(nt):
                # both heads' q (or k) in one stacked M=128 chain; k chains
                # use the otherwise-idle "sc" banks so the gp rotation never
                # gates phase 0
                qp = ps.tile((P, 512), FP32, tag="gp", bufs=2, name="qp")
                for cc in range(CT // 2):
                    nc.tensor.matmul(
                        qp[:],
                        wqs_sb[:, ds(2 * cc, 2), :],
                        x8_sb[:, ds(2 * cc, 2), ts(nt, 512)],
                        start=(cc == 0),
                        stop=(cc == CT // 2 - 1),
                        perf_mode=DR,
                    )
                nc.scalar.copy(qb_sb[:, ts(nt, 512)], qp[:])
                kp = ps.tile((P, 512), FP32, tag="gp", bufs=2, name="kp")
                for cc in range(CT // 2):
                    nc.tensor.matmul(
                        kp[:],
                        wks_sb[:, ds(2 * cc, 2), :],
                        x8_sb[:, ds(2 * cc, 2), ts(nt, 512)],
                        start=(cc == 0),
                        stop=(cc == CT // 2 - 1),
                        perf_mode=DR,
                    )
                nc.vector.tensor_copy(kb_sb[:, ts(nt, 512)], kp[:])

            def sc_exp(h, it):
                # h0 in array rows 0-63, h1 in rows 64-127 (row-group MMs)
                lo, hi = (0, D) if h == 0 else (D, P)
                for j2 in range(2):
                    sp = ps.tile((P, 2, 512), FP32, tag="sc", bufs=3, name="sp")
                    for jh in range(2):
                        nc.tensor.matmul(
                            sp[:, jh],
                            qb_sb[lo:hi, ts(it, P)],
                            kb_sb[lo:hi, ds(j2 * 1024 + jh * 512, 512)],
                        )
                    nc.scalar.activation(
                        p_sb[:, h, it, ds(2 * j2, 2)],
                        sp[:],
                        mybir.ActivationFunctionType.Exp,
                        bias=eb_sb[:],
                        scale=ACT_SCALE,
                        accum_out=sums[:, h, it, ds(j2, 1)],
                    )

            def vt_mm(h, it):
                vp = ps.tile((P, 512), FP32, tag="gp", bufs=2, name="vp")
                for cc in range(CT // 2):
                    nc.tensor.matmul(
                        vp[:],
                        x8_sb[:, ds(2 * cc, 2), ts(it, P)],
                        wv_sb[:, h, ds(2 * cc, 2), :],
                        start=(cc == 0),
                        stop=(cc == CT // 2 - 1),
                        perf_mode=DR,
                    )
                return vp

            def rinv_it(h, it):
                nc.vector.tensor_add(
                    rsum[:, h, ds(it, 1)],
                    sums[:, h, it, ds(0, 1)],
                    sums[:, h, it, ds(1, 1)],
                )
                nc.vector.reciprocal(rinv[:, h, ds(it, 1)], rsum[:, h, ds(it, 1)])

            def ctx_chunk(h, et, jt, dma_out):
                cp = ps.tile((P, 512), FP32, tag="gp", bufs=2, name="cp")
                for kk in range(IT // 2):
                    nc.tensor.matmul(
                        cp[:],
                        vt8_sb[:, h, ds(2 * kk, 2), ts(et, P)],
                        p_sb[:, h, ds(2 * kk, 2), jt],
                        start=(kk == 0),
                        stop=(kk == IT // 2 - 1),
                        perf_mode=DR,
                    )
                nc.vector.tensor_add(
                    outa[:, et, ts(jt, 512)], outa[:, et, ts(jt, 512)], cp[:]
                )
                if dma_out:
                    eng = qs[(et * JT + jt) % 3]
                    eng.dma_start(
                        out_d[ts(et, P), ts(jt, 512)], outa[:, et, ts(jt, 512)]
                    )

            # phase 0: stacked qk projection (both heads per chain)
            for nt in range(JT):
                qk_nt(nt)

            # phase 1: ACT-bound exp h0; PE also precomputes v^T h1
            for it in range(IT):
                sc_exp(0, it)
                vp0 = vt_mm(0, it)
                rinv_it(0, it)
                nc.vector.tensor_scalar_mul(
                    vt8_sb[:, 0, it], vp0[:], rinv[:, 0, ds(it, 1)]
                )
                vp1 = vt_mm(1, it)
                nc.vector.tensor_copy(v1_sb[:, it], vp1[:])
                if it == 14:
                    nc.vector.tensor_scalar_mul(outa[:, 0], xb_sb[:, 0], rs_sb[:])

            # phase 2: ACT exp h1; PE ctx h0 lagged one it behind the scores
            for it in range(IT):
                sc_exp(1, it)
                rinv_it(1, it)
                nc.vector.tensor_scalar_mul(
                    vt8_sb[:, 1, it], v1_sb[:, it], rinv[:, 1, ds(it, 1)]
                )
                if it in (2, 6, 10):
                    ct = it // 4 + 1
                    nc.vector.tensor_scalar_mul(outa[:, ct], xb_sb[:, ct], rs_sb[:])
                if it >= 1:
                    t = it - 1
                    ctx_chunk(0, t // JT, t % JT, dma_out=False)
            ctx_chunk(0, 3, 3, dma_out=False)

            # phase 3: ctx h1, ACT idle
            for et in range(ET):
                for jt in range(JT):
                    ctx_chunk(1, et, jt, dma_out=True)

    nc.finalize()
    return nc


def kernel(x, Wq, bq, Wk, bk, Wv, bv):
    global _NC_CACHE, LAST_EXEC_NS, LAST_MEAN_EXEC_NS
    x = np.ascontiguousarray(np.asarray(x, dtype=np.float32))
    Wq = np.asarray(Wq, dtype=np.float32)
    Wk = np.asarray(Wk, dtype=np.float32)
    Wv = np.asarray(Wv, dtype=np.float32)
    scale = np.float32(D ** -0.5)

    if _NC_CACHE is None:
        _NC_CACHE = _build()
    nc = _NC_CACHE

    x8 = x.astype(E4M3)
    xb = x.astype(ml_dtypes.bfloat16)

    # core c -> batch c//2, head pair c%2 (heads 2p, 2p+1)
    # wqs/wks: both heads of the pair stacked on the output axis -> [C, 128]
    wqs_pair = []
    wks_pair = []
    wv_pair = []
    for pair in range(2):
        hs = [2 * pair, 2 * pair + 1]
        wqs_pair.append(
            np.ascontiguousarray(
                np.concatenate(
                    [Wq[h].T * (QK_SCALE * scale) for h in hs], axis=1
                ).astype(E4M3)
            )
        )
        wks_pair.append(
            np.ascontiguousarray(
                np.concatenate([Wk[h].T * QK_SCALE for h in hs], axis=1).astype(E4M3)
            )
        )
        wv_pair.append(
            np.ascontiguousarray(
                (np.stack([Wv[h].T for h in hs]) * V_SCALE).astype(E4M3)
            )
        )

    in_maps = []
    for c in range(8):
        b, pair = c // 2, c % 2
        in_maps.append(
            {
                "x8": x8[b],
                "xb": xb[b],
                "wqs": wqs_pair[pair],
                "wks": wks_pair[pair],
                "wv": wv_pair[pair],
                "rs": np.full(
                    (P, 1), 2.0 * GAMMA if pair == 0 else 0.0, dtype=np.float32
                ),
            }
        )

    res = bass_utils.run_bass_kernel_spmd(nc, in_maps, core_ids=list(range(8)))
    LAST_EXEC_NS = res.exec_time_ns
    LAST_MEAN_EXEC_NS = res.mean_exec_time_ns

    out = np.empty((B, C, W), dtype=np.float32)
    inv_g = np.float32(1.0 / GAMMA)
    for b in range(B):
        out[b] = (res.results[2 * b]["out"] + res.results[2 * b + 1]["out"]) * inv_g
    return out
